# revision 15
# baseline (speedup 1.0000x reference)
"""CaptioningRNN (LSTM + tiny spatial attention) Trainium2 kernel, v3.

Contract: kernel(**inputs) takes FULL inputs (numpy), returns FULL output
(N, T, H) float32.  Internally: data-parallel over batch N across 8
NeuronCores (16 sequences per core, zero cross-core traffic).

Per-core algorithm (v3 — phase 0a interleaved into the recurrence):
  prologue: B[n,m,:] = A[n,:,m] @ Wattn (PE, bf16) -> SBUF resident,
            plus P blocks 0,1 (P[t] = x_t @ Wx (+b), gate-permuted cols).
  steps:    512 sequential LSTM steps.  Per step:
    - gates PSUM bank jt holds [i|f|o|g] for H-slice jt
    - h_t -> hT via 4 DMA-XBAR transposes (off the PE critical path)
    - softmax without Exp: e^s = (1+tanh(s/2))/(1-tanh(s/2))
    - 2 matmuls of P block (t//8 + 2) interleaved into the PE idle
      window while the softmax chain runs (just-in-time x@Wx)
    - PSUM gate buffers rotate partitions 0/32/64 by step mod 3
"""

import sys
import numpy as np

sys.path.insert(0, "/opt/trn_rl_repo")

import ml_dtypes

BF16 = ml_dtypes.bfloat16

N, T, D, H, M = 128, 512, 512, 512, 16
NCORES = 8
NL = N // NCORES          # 16 sequences per core
KC = 4                    # 512 = 4 chunks of 128 (contraction dims)
J = 4 * H                 # 2048 gate columns
TB = 8                    # time steps per phase-0 row block
NG = 2                    # sequence groups of 8 for the B contraction

_CACHE = {}


def build(t_steps=T, has_bias=False):
    from concourse import bacc, mybir
    import concourse.tile as tile

    f32 = mybir.dt.float32
    bf16 = mybir.dt.bfloat16
    mult = mybir.AluOpType.mult
    add = mybir.AluOpType.add
    AF = mybir.ActivationFunctionType
    AX = mybir.AxisListType.X

    rb = NL * t_steps // 128

    nc = bacc.Bacc("TRN2", target_bir_lowering=False, debug=False,
                   num_devices=NCORES)

    # ---- I/O -----------------------------------------------------------
    xs = nc.dram_tensor("xs", [rb, 128, KC, 128], bf16, kind="ExternalInput")
    at_d = nc.dram_tensor("at", [128, KC, NL, M], bf16, kind="ExternalInput")
    wx_d = nc.dram_tensor("wx", [128, KC, J], bf16, kind="ExternalInput")
    wh_d = nc.dram_tensor("wh", [128, KC, J], bf16, kind="ExternalInput")
    wa_d = nc.dram_tensor("wa", [128, KC, J], bf16, kind="ExternalInput")
    h0_d = nc.dram_tensor("h0t", [128, KC, NL], bf16, kind="ExternalInput")
    c0_d = nc.dram_tensor("c0", [NL, H], f32, kind="ExternalInput")
    id_d = nc.dram_tensor("ident", [NL, NL], bf16, kind="ExternalInput")
    oc_d = nc.dram_tensor("ones_col", [128, 1], bf16, kind="ExternalInput")
    m0_d = nc.dram_tensor("m0", [128, NG, NL], bf16, kind="ExternalInput")
    if has_bias:
        b_d = nc.dram_tensor("bvec", [1, J], f32, kind="ExternalInput")
    p_d = nc.dram_tensor("pbuf", [rb, 128, J], bf16)
    out_d = nc.dram_tensor("out", [NL, t_steps, H], bf16,
                           kind="ExternalOutput")

    half_inv_sqrt_h = float(0.5 / np.sqrt(H))

    from contextlib import ExitStack
    with tile.TileContext(nc) as tc, ExitStack() as stack:
        # ---- persistent constants -------------------------------------
        cpool = stack.enter_context(tc.tile_pool(name="consts", bufs=1))
        wh_s = cpool.tile([128, KC, J], bf16)
        wx_s = cpool.tile([128, KC, J], bf16)
        at_s = cpool.tile([128, KC, NL, M], bf16)
        b_s = cpool.tile([128, NG, J], bf16)          # B (attention basis)
        h0_s = cpool.tile([128, KC, NL], bf16)
        id_s = cpool.tile([NL, NL], bf16)
        oc_s = cpool.tile([128, 1], bf16)
        m0_s = cpool.tile([128, NG, NL], bf16)
        nc.sync.dma_start(out=wh_s[:, :, :], in_=wh_d.ap()[:, :, :])
        nc.sync.dma_start(out=wx_s[:, :, :], in_=wx_d.ap()[:, :, :])
        nc.sync.dma_start(out=at_s[:, :, :, :], in_=at_d.ap()[:, :, :, :])
        nc.sync.dma_start(out=h0_s[:, :, :], in_=h0_d.ap()[:, :, :])
        nc.sync.dma_start(out=id_s[:, :], in_=id_d.ap()[:, :])
        nc.sync.dma_start(out=oc_s[:, :], in_=oc_d.ap()[:, :])
        nc.sync.dma_start(out=m0_s[:, :, :], in_=m0_d.ap()[:, :, :])

        # ---- persistent PSUM ------------------------------------------
        pp = stack.enter_context(tc.tile_pool(name="ppsum", bufs=1,
                                              space="PSUM"))
        # separate tiles per gate bank: byte-range dep tracking linearizes
        # multi-partition slices, so slices of one big tile would
        # false-conflict across banks and serialize the pipeline
        # psa rotates over 3 partition slots (0/32/64) so a step's gate
        # writes only WAR against step t-3's activation reads
        psa_b = [pp.tile([80, 512], f32, name=f"psa{i}") for i in range(4)]
        psz_r = [pp.tile([1, NL, M], f32, name=f"psz{i}") for i in range(2)]
        pX = pp.tile([128, 512], f32, name="p0a")     # phase-0a accum bank
        wc_p = pp.tile([128, 2, 2], bf16, name="wcol")
        wcol = [wc_p[:, 0, 0:1], wc_p[:, 1, 0:1]]

        # ---- pools -----------------------------------------------------
        with tc.tile_pool(name="prolog", bufs=1) as p0c, \
             tc.tile_pool(name="xblk", bufs=3) as xp, \
             tc.tile_pool(name="pout", bufs=3) as pop, \
             tc.tile_pool(name="state", bufs=3) as stp, \
             tc.tile_pool(name="work", bufs=3) as wk, \
             tc.tile_pool(name="pin", bufs=3) as pin, \
             tc.tile_pool(name="hout", bufs=3) as hop:

            wa_s = p0c.tile([128, KC, J], bf16)
            nc.sync.dma_start(out=wa_s[:, :, :], in_=wa_d.ap()[:, :, :])

            if has_bias:
                bf_s = p0c.tile([1, J], f32)
                nc.sync.dma_start(out=bf_s[:, :], in_=b_d.ap()[:, :])
                bb_s = p0c.tile([1, J], bf16)
                nc.vector.tensor_copy(bb_s[:, :], bf_s[:, :])
                or_s = p0c.tile([1, 128], bf16)
                nc.vector.memset(or_s[:, :], 1.0)
                brep = p0c.tile([128, J], bf16)
                for jt in range(4):
                    nc.tensor.matmul(pX[:, :], or_s[:, :],
                                     bb_s[:, jt * 512:(jt + 1) * 512],
                                     start=True, stop=True)
                    nc.vector.tensor_copy(brep[:, jt * 512:(jt + 1) * 512],
                                          pX[:, :])

            # ---- phase 0b: B[(i,m), g, :] = sum_h A[g8+i, h, m] Wattn[h, :]
            for g in range(NG):
                for jt in range(4):
                    for kc in range(KC):
                        nc.tensor.matmul(
                            pX[:, :],
                            at_s[:, kc, g * 8:(g + 1) * 8, :],
                            wa_s[:, kc, jt * 512:(jt + 1) * 512],
                            start=(kc == 0), stop=(kc == KC - 1))
                    sl = slice(jt * 512, (jt + 1) * 512)
                    if jt in (1, 3):
                        nc.scalar.copy(b_s[:, g, sl], pX[:, :])
                    else:
                        nc.vector.tensor_copy(b_s[:, g, sl], pX[:, :])

            # ---- phase 0a machinery (P = x @ Wx (+b)) ------------------
            def p0a_fetch_x(b_i):
                xt = xp.tile([128, KC, 128], bf16, tag="xt")
                nc.sync.dma_start(out=xt[:, :, :], in_=xs.ap()[b_i, :, :, :])
                return xt

            def p0a_matmul(xt, jt, kc):
                nc.tensor.matmul(
                    pX[:, :], xt[:, kc, :],
                    wx_s[:, kc, jt * 512:(jt + 1) * 512],
                    start=(kc == 0), stop=(kc == KC - 1))

            def p0a_flush(b_i, jt):
                po = pop.tile([128, 512], bf16, tag="po")
                sl = slice(jt * 512, (jt + 1) * 512)
                if has_bias:
                    nc.vector.tensor_tensor(po[:, :], pX[:, :],
                                            brep[:, sl], add)
                elif jt in (1, 3):
                    nc.scalar.copy(po[:, :], pX[:, :])
                else:
                    nc.vector.tensor_copy(po[:, :], pX[:, :])
                nc.scalar.dma_start(out=p_d.ap()[b_i, :, sl], in_=po[:, :])

            def p0a_block(b_i):
                xt = p0a_fetch_x(b_i)
                for jt in range(4):
                    for kc in range(KC):
                        p0a_matmul(xt, jt, kc)
                    p0a_flush(b_i, jt)

            # prologue: blocks 0 and 1
            for b_i in range(min(2, rb)):
                p0a_block(b_i)

            # ---- phase 1: recurrence ----------------------------------
            c_cur = []
            for jt in range(4):
                c_j = stp.tile([NL, 128], f32, tag=f"c{jt}", name="c_j")
                nc.sync.dma_start(out=c_j[:, :],
                                  in_=c0_d.ap()[:, jt * 128:(jt + 1) * 128])
                c_cur.append(c_j)
            hT_cur = h0_s
            xt_cur = None

            for t in range(t_steps):
                p0 = (t % 3) * 32
                psz = psz_r[t % 2]

                # just-in-time phase-0a: block b_i = t//8 + 2, two matmuls
                # per step (jt chunk = (t%8)//2, kc pair = (t%8)%2)
                b_i = t // TB + 2
                sub = t % TB
                do_p0a = b_i < rb
                if do_p0a and sub == 0:
                    xt_cur = p0a_fetch_x(b_i)

                p_t = pin.tile([NL, J], bf16, tag="pt")
                bb, tt = divmod(t, TB)
                nc.scalar.dma_start(out=p_t[:, :],
                                    in_=p_d.ap()[bb, tt * NL:(tt + 1) * NL, :])

                # -- attention scores: s2 = A * hT (broadcast over m)
                s2 = [wk.tile([128, NL, M], bf16, tag=f"s2k{kc}",
                              name="s2_k") for kc in range(KC)]
                for kc in range(KC):
                    eng = nc.vector if kc == KC - 1 else nc.gpsimd
                    eng.tensor_tensor(
                        s2[kc][:, :, :], at_s[:, kc, :, :],
                        hT_cur[:, kc, :, None].broadcast_to([128, NL, M]),
                        mult)

                # -- Wh matmuls (kc 0-2) interleaved with the colsum
                for kc in range(KC):
                    if kc < KC - 1:
                        # Wh for the last chunk is held back to fill the
                        # PE idle window while the softmax chain runs
                        for jt in range(4):
                            nc.tensor.matmul(
                                psa_b[jt][p0:p0 + 16, :],
                                hT_cur[:, kc, :],
                                wh_s[:, kc, jt * 512:(jt + 1) * 512],
                                start=(kc == 0), stop=False)
                    if kc > 0:
                        # colsum for chunk kc-1 (s2 ready by now)
                        nc.tensor.matmul(psz[:, :, :], oc_s[:, :],
                                         s2[kc - 1][:, :, :],
                                         start=(kc == 1), stop=False)
                nc.tensor.matmul(psz[:, :, :], oc_s[:, :], s2[KC - 1][:, :, :],
                                 start=False, stop=True)

                # -- P inject + Wh kc3 (PE fillers while softmax runs)
                for jt in range(4):
                    nc.tensor.matmul(psa_b[jt][p0:p0 + 16, :],
                                     id_s[:, :],
                                     p_t[:, jt * 512:(jt + 1) * 512],
                                     start=False, stop=False)
                for jt in range(4):
                    nc.tensor.matmul(
                        psa_b[jt][p0:p0 + 16, :],
                        hT_cur[:, KC - 1, :],
                        wh_s[:, KC - 1, jt * 512:(jt + 1) * 512],
                        start=False, stop=False)
                # -- phase-0a fillers (2 matmuls of a future P block)
                if do_p0a:
                    jt0a = sub // 2
                    for kc in (0, 1) if sub % 2 == 0 else (2, 3):
                        p0a_matmul(xt_cur, jt0a, kc)
                    if sub % 2 == 1:
                        p0a_flush(b_i, jt0a)

                # -- softmax via tanh: e^s = (1+u)/(1-u), u = tanh(s/2)
                u_t = wk.tile([1, NL, M], f32, tag="u")
                nc.scalar.activation(u_t[:, :, :], psz[:, :, :], AF.Tanh,
                                     scale=half_inv_sqrt_h)
                den = wk.tile([1, NL, M], f32, tag="den")
                nc.vector.tensor_scalar(den[:, :, :], u_t[:, :, :],
                                        -1.0, 1.0, mult, add)
                rden = wk.tile([1, NL, M], f32, tag="rden")
                nc.vector.reciprocal_approx_fast(rden[:, :, :], den[:, :, :])
                r_t = wk.tile([1, NL, M], f32, tag="r")
                nc.vector.tensor_scalar(r_t[:, :, :], rden[:, :, :],
                                        2.0, -1.0, mult, add)
                ssum = wk.tile([1, NL, 1], f32, tag="ssum")
                nc.vector.tensor_reduce(ssum[:, :, :], r_t[:, :, :], AX, add)
                rsum = wk.tile([1, NL, 1], f32, tag="rsum")
                nc.vector.reciprocal_approx_fast(rsum[:, :, :], ssum[:, :, :])
                w_t = wk.tile([1, NL, M], bf16, tag="w")
                nc.vector.tensor_tensor(
                    w_t[:, :, :], r_t[:, :, :],
                    rsum[:, :, :].broadcast_to([1, NL, M]), mult)

                # -- w to partitions, block-diag stationary S_g
                s_g = wk.tile([128, NG, NL], bf16, tag="sg_w")
                for g in range(NG):
                    nc.tensor.transpose(wcol[g],
                                        w_t[0:1, g * 8:(g + 1) * 8, :],
                                        oc_s[0:1, 0:1])
                    nc.vector.tensor_tensor(
                        s_g[:, g, :], m0_s[:, g, :],
                        wcol[g].broadcast_to([128, NL]), mult)

                # -- attention contribution: psa += S_g^T . B_g, bank stops
                h_out = hop.tile([NL, H], bf16, tag="h")
                if t < t_steps - 1:
                    hT_next = stp.tile([128, KC, NL], bf16, tag="hT",
                                       name="hT_next")
                else:
                    hT_next = None
                c_nxt = [None] * 4
                sgs = []

                def h_finish(pj, psg):
                    pcs = slice(pj * 128, (pj + 1) * 128)
                    tc_t = wk.tile([NL, 128], bf16, tag=f"tc{pj}")
                    nc.scalar.activation(tc_t[:, :], c_nxt[pj][:, :], AF.Tanh)
                    nc.vector.tensor_tensor(h_out[:, pcs],
                                            psg[:, 256:384], tc_t[:, :],
                                            mult)

                for jt in range(4):
                    sl = slice(jt * 512, (jt + 1) * 512)
                    nc.tensor.matmul(psa_b[jt][p0:p0 + 16, :], s_g[:, 0, :],
                                     b_s[:, 0, sl], start=False, stop=False)
                    nc.tensor.matmul(psa_b[jt][p0:p0 + 16, :], s_g[:, 1, :],
                                     b_s[:, 1, sl], start=False, stop=True)
                    # bank jt complete: gates for H-slice jt
                    sg_t = wk.tile([NL, 384], bf16, tag=f"sg{jt}")
                    nc.scalar.activation(sg_t[:, :],
                                         psa_b[jt][p0:p0 + 16, 0:384],
                                         AF.Sigmoid)
                    tg_t = wk.tile([NL, 128], bf16, tag=f"tg{jt}")
                    nc.scalar.activation(tg_t[:, :],
                                         psa_b[jt][p0:p0 + 16, 384:512],
                                         AF.Tanh)
                    # c update for this slice
                    t1 = wk.tile([NL, 128], f32, tag=f"t1{jt}")
                    nc.gpsimd.tensor_tensor(t1[:, :], sg_t[:, 128:256],
                                            c_cur[jt][:, :], mult)
                    t2 = wk.tile([NL, 128], bf16, tag=f"t2{jt}")
                    nc.vector.tensor_tensor(t2[:, :], sg_t[:, 0:128],
                                            tg_t[:, :], mult)
                    c_nj = stp.tile([NL, 128], f32, tag=f"c{jt}", name="c_nj")
                    nc.vector.tensor_tensor(c_nj[:, :], t1[:, :],
                                            t2[:, :], add)
                    c_nxt[jt] = c_nj
                    sgs.append((jt, sg_t))
                    # tanh(c) lagged by one bank so the Act queue never
                    # stalls waiting for the c chain
                    if jt > 0:
                        h_finish(*sgs[jt - 1])
                h_finish(*sgs[3])

                if hT_next is not None:
                    # one XBAR transpose produces all of hT: the XBAR tiles
                    # land as hT[p, kc, n] = h_out[n, kc*128+p]
                    nc.sync.dma_start(out=hT_next[:, :, :], in_=h_out[:, :],
                                      transpose=True)
                nc.sync.dma_start(out=out_d.ap()[:, t, :], in_=h_out[:, :])

                hT_cur = hT_next
                c_cur = c_nxt

    nc.compile()
    return nc


def _perm_cols(w):
    """Permute gate columns: new bank jt = [i_jt | f_jt | o_jt | g_jt]."""
    # w: (..., 4H) with original layout [i(512) | f | o | g]
    w4 = w.reshape(w.shape[:-1] + (4, 4, 128))   # (..., gate, jt, col)
    return np.ascontiguousarray(
        np.moveaxis(w4, -3, -2).reshape(w.shape))  # (..., jt, gate, col)


def _stage_inputs(x, A, Wx, Wh, Wattn, b, t_steps=T):
    """Shard + lay out inputs per core (host-side numpy staging)."""
    rb = NL * t_steps // 128
    h0 = A.mean(axis=(2, 3)).astype(np.float32)          # (N, H)
    ident = np.eye(NL, dtype=BF16)
    ones_col = np.ones((128, 1), dtype=BF16)
    m0 = np.zeros((128, NG, NL), dtype=BF16)
    for g in range(NG):
        for i in range(8):
            m0[i * 16:(i + 1) * 16, g, g * 8 + i] = 1

    def wlay(w):
        return np.ascontiguousarray(
            _perm_cols(w).astype(BF16).reshape(KC, 128, J).transpose(1, 0, 2))

    wxs, whs, was = wlay(Wx), wlay(Wh), wlay(Wattn)
    bvec = np.ascontiguousarray(_perm_cols(b.astype(np.float32))
                                .reshape(1, J))

    maps = []
    for k in range(NCORES):
        ns = slice(k * NL, (k + 1) * NL)
        x_sh = x[ns, :t_steps].astype(BF16)              # (NL, t, D)
        xT = x_sh.transpose(2, 0, 1).reshape(KC, 128, NL, rb, TB)
        xs_st = np.ascontiguousarray(
            xT.transpose(3, 1, 0, 4, 2).reshape(rb, 128, KC, 128))
        A_sh = A[ns].reshape(NL, H, M).astype(BF16)
        at_st = np.ascontiguousarray(
            A_sh.transpose(1, 0, 2).reshape(KC, 128, NL, M)
            .transpose(1, 0, 2, 3))
        h0_sh = h0[ns]                                    # (NL, H)
        h0t = np.ascontiguousarray(
            h0_sh.T.astype(BF16).reshape(KC, 128, NL).transpose(1, 0, 2))
        m = {
            "xs": xs_st, "at": at_st, "wx": wxs, "wh": whs, "wa": was,
            "h0t": h0t, "c0": np.ascontiguousarray(h0_sh),
            "ident": ident, "ones_col": ones_col, "m0": m0,
        }
        if np.any(b != 0):
            m["bvec"] = bvec
        maps.append(m)
    return maps


def _get_nc(has_bias, t_steps=T):
    key = (has_bias, t_steps)
    if key not in _CACHE:
        _CACHE[key] = build(t_steps=t_steps, has_bias=has_bias)
    return _CACHE[key]


def run_cores(x, A, Wx, Wh, Wattn, b, t_steps=T, trace=False):
    from concourse.bass_utils import run_bass_kernel_spmd
    maps = _stage_inputs(x, A, Wx, Wh, Wattn, b, t_steps=t_steps)
    has_bias = "bvec" in maps[0]
    nc = _get_nc(has_bias, t_steps)
    res = run_bass_kernel_spmd(nc, maps, list(range(NCORES)), trace=trace)
    out = np.concatenate(
        [np.asarray(res.results[k]["out"], dtype=np.float32)
         for k in range(NCORES)], axis=0)
    return out, res


def kernel(x, A, Wx, Wh, Wattn, b):
    x = np.asarray(x, dtype=np.float32)
    A = np.asarray(A, dtype=np.float32)
    out, _ = run_cores(x, A,
                       np.asarray(Wx, dtype=np.float32),
                       np.asarray(Wh, dtype=np.float32),
                       np.asarray(Wattn, dtype=np.float32),
                       np.asarray(b, dtype=np.float32))
    return out


# revision 20
# speedup vs baseline: 1.2533x; 1.2533x over previous
"""CaptioningRNN (LSTM + tiny spatial attention) Trainium2 kernel, v3.

Contract: kernel(**inputs) takes FULL inputs (numpy), returns FULL output
(N, T, H) float32.  Internally: data-parallel over batch N across 8
NeuronCores (16 sequences per core, zero cross-core traffic).

Per-core algorithm (v3 — phase 0a interleaved into the recurrence):
  prologue: B[n,m,:] = A[n,:,m] @ Wattn (PE, bf16) -> SBUF resident,
            plus P blocks 0,1 (P[t] = x_t @ Wx (+b), gate-permuted cols).
  steps:    512 sequential LSTM steps.  Per step:
    - gates PSUM bank jt holds [i|f|o|g] for H-slice jt
    - h_t -> hT via 4 DMA-XBAR transposes (off the PE critical path)
    - softmax without Exp: e^s = (1+tanh(s/2))/(1-tanh(s/2))
    - 2 matmuls of P block (t//8 + 2) interleaved into the PE idle
      window while the softmax chain runs (just-in-time x@Wx)
    - PSUM gate buffers rotate partitions 0/32/64 by step mod 3
"""

import sys
import numpy as np

sys.path.insert(0, "/opt/trn_rl_repo")

import ml_dtypes

BF16 = ml_dtypes.bfloat16

N, T, D, H, M = 128, 512, 512, 512, 16
NCORES = 8
NL = N // NCORES          # 16 sequences per core
KC = 4                    # 512 = 4 chunks of 128 (contraction dims)
J = 4 * H                 # 2048 gate columns
TB = 8                    # time steps per phase-0 row block
NG = 2                    # sequence groups of 8 for the B contraction

_CACHE = {}


def build(t_steps=T, has_bias=False):
    from concourse import bacc, mybir
    import concourse.tile as tile

    f32 = mybir.dt.float32
    bf16 = mybir.dt.bfloat16
    mult = mybir.AluOpType.mult
    add = mybir.AluOpType.add
    AF = mybir.ActivationFunctionType
    AX = mybir.AxisListType.X

    rb = NL * t_steps // 128

    nc = bacc.Bacc("TRN2", target_bir_lowering=False, debug=False,
                   num_devices=NCORES)

    # ---- I/O -----------------------------------------------------------
    xs = nc.dram_tensor("xs", [rb, 128, KC, 128], bf16, kind="ExternalInput")
    at_d = nc.dram_tensor("at", [128, KC, NL, M], bf16, kind="ExternalInput")
    wx_d = nc.dram_tensor("wx", [128, KC, J], bf16, kind="ExternalInput")
    wh_d = nc.dram_tensor("wh", [128, KC, J], bf16, kind="ExternalInput")
    wa_d = nc.dram_tensor("wa", [128, KC, J], bf16, kind="ExternalInput")
    h0_d = nc.dram_tensor("h0t", [128, KC, NL], bf16, kind="ExternalInput")
    c0_d = nc.dram_tensor("c0", [NL, H], f32, kind="ExternalInput")
    id_d = nc.dram_tensor("ident", [NL, NL], bf16, kind="ExternalInput")
    oc_d = nc.dram_tensor("ones_col", [128, 1], bf16, kind="ExternalInput")
    m0_d = nc.dram_tensor("m0", [128, NG, NL], bf16, kind="ExternalInput")
    if has_bias:
        b_d = nc.dram_tensor("bvec", [1, J], f32, kind="ExternalInput")
    p_d = nc.dram_tensor("pbuf", [rb, 128, J], bf16)
    out_d = nc.dram_tensor("out", [NL, t_steps, H], bf16,
                           kind="ExternalOutput")

    half_inv_sqrt_h = float(0.5 / np.sqrt(H))

    from contextlib import ExitStack
    with tile.TileContext(nc) as tc, ExitStack() as stack:
        # ---- persistent constants -------------------------------------
        cpool = stack.enter_context(tc.tile_pool(name="consts", bufs=1))
        wh_s = cpool.tile([128, KC, J], bf16)
        wx_s = cpool.tile([128, KC, J], bf16)
        at_s = cpool.tile([128, KC, NL, M], bf16)
        b_s = cpool.tile([128, NG, J], bf16)          # B (attention basis)
        h0_s = cpool.tile([128, KC, NL], bf16)
        id_s = cpool.tile([NL, NL], bf16)
        oc_s = cpool.tile([128, 1], bf16)
        m0_s = cpool.tile([128, NG, NL], bf16)
        nc.sync.dma_start(out=wh_s[:, :, :], in_=wh_d.ap()[:, :, :])
        nc.sync.dma_start(out=wx_s[:, :, :], in_=wx_d.ap()[:, :, :])
        nc.sync.dma_start(out=at_s[:, :, :, :], in_=at_d.ap()[:, :, :, :])
        nc.sync.dma_start(out=h0_s[:, :, :], in_=h0_d.ap()[:, :, :])
        nc.sync.dma_start(out=id_s[:, :], in_=id_d.ap()[:, :])
        nc.sync.dma_start(out=oc_s[:, :], in_=oc_d.ap()[:, :])
        nc.sync.dma_start(out=m0_s[:, :, :], in_=m0_d.ap()[:, :, :])

        # ---- persistent PSUM ------------------------------------------
        pp = stack.enter_context(tc.tile_pool(name="ppsum", bufs=1,
                                              space="PSUM"))
        # separate tiles per gate bank: byte-range dep tracking linearizes
        # multi-partition slices, so slices of one big tile would
        # false-conflict across banks and serialize the pipeline
        # psa rotates over 3 partition slots (0/32/64) so a step's gate
        # writes only WAR against step t-3's activation reads
        psa_b = [pp.tile([80, 512], f32, name=f"psa{i}") for i in range(4)]
        psz_r = [pp.tile([1, NL, M], f32, name=f"psz{i}") for i in range(2)]
        pX = pp.tile([128, 512], f32, name="p0a")     # phase-0a accum bank
        # transpose staging: h-chunk transposes + w-column transposes share
        # one bank; all writers are transposes (assign, never accumulate)
        tp = pp.tile([128, KC, NL + 2], bf16, name="tpose")
        wcol = [tp[:, 0, NL:NL + 1], tp[:, 1, NL:NL + 1]]

        # ---- pools -----------------------------------------------------
        with tc.tile_pool(name="prolog", bufs=1) as p0c, \
             tc.tile_pool(name="xblk", bufs=3) as xp, \
             tc.tile_pool(name="pout", bufs=3) as pop, \
             tc.tile_pool(name="state", bufs=3) as stp, \
             tc.tile_pool(name="work", bufs=3) as wk, \
             tc.tile_pool(name="pin", bufs=3) as pin, \
             tc.tile_pool(name="hout", bufs=3) as hop:

            wa_s = p0c.tile([128, KC, J], bf16)
            nc.sync.dma_start(out=wa_s[:, :, :], in_=wa_d.ap()[:, :, :])

            if has_bias:
                bf_s = p0c.tile([1, J], f32)
                nc.sync.dma_start(out=bf_s[:, :], in_=b_d.ap()[:, :])
                bb_s = p0c.tile([1, J], bf16)
                nc.vector.tensor_copy(bb_s[:, :], bf_s[:, :])
                or_s = p0c.tile([1, 128], bf16)
                nc.vector.memset(or_s[:, :], 1.0)
                brep = p0c.tile([128, J], bf16)
                for jt in range(4):
                    nc.tensor.matmul(pX[:, :], or_s[:, :],
                                     bb_s[:, jt * 512:(jt + 1) * 512],
                                     start=True, stop=True)
                    nc.vector.tensor_copy(brep[:, jt * 512:(jt + 1) * 512],
                                          pX[:, :])

            # ---- phase 0b: B[(i,m), g, :] = sum_h A[g8+i, h, m] Wattn[h, :]
            for g in range(NG):
                for jt in range(4):
                    for kc in range(KC):
                        nc.tensor.matmul(
                            pX[:, :],
                            at_s[:, kc, g * 8:(g + 1) * 8, :],
                            wa_s[:, kc, jt * 512:(jt + 1) * 512],
                            start=(kc == 0), stop=(kc == KC - 1))
                    sl = slice(jt * 512, (jt + 1) * 512)
                    if jt in (1, 3):
                        nc.scalar.copy(b_s[:, g, sl], pX[:, :])
                    else:
                        nc.vector.tensor_copy(b_s[:, g, sl], pX[:, :])

            # ---- phase 0a machinery (P = x @ Wx (+b)) ------------------
            def p0a_fetch_x(b_i):
                xt = xp.tile([128, KC, 128], bf16, tag="xt")
                nc.sync.dma_start(out=xt[:, :, :], in_=xs.ap()[b_i, :, :, :])
                return xt

            def p0a_matmul(xt, jt, kc):
                nc.tensor.matmul(
                    pX[:, :], xt[:, kc, :],
                    wx_s[:, kc, jt * 512:(jt + 1) * 512],
                    start=(kc == 0), stop=(kc == KC - 1))

            def p0a_flush(b_i, jt):
                po = pop.tile([128, 512], bf16, tag="po")
                sl = slice(jt * 512, (jt + 1) * 512)
                if has_bias:
                    nc.vector.tensor_tensor(po[:, :], pX[:, :],
                                            brep[:, sl], add)
                elif jt in (1, 3):
                    nc.scalar.copy(po[:, :], pX[:, :])
                else:
                    nc.vector.tensor_copy(po[:, :], pX[:, :])
                nc.scalar.dma_start(out=p_d.ap()[b_i, :, sl], in_=po[:, :])

            def p0a_block(b_i):
                xt = p0a_fetch_x(b_i)
                for jt in range(4):
                    for kc in range(KC):
                        p0a_matmul(xt, jt, kc)
                    p0a_flush(b_i, jt)

            # prologue: blocks 0 and 1
            for b_i in range(min(2, rb)):
                p0a_block(b_i)

            # ---- phase 1: recurrence ----------------------------------
            c_cur = []
            for jt in range(4):
                c_j = stp.tile([NL, 128], f32, tag=f"c{jt}", name="c_j")
                nc.sync.dma_start(out=c_j[:, :],
                                  in_=c0_d.ap()[:, jt * 128:(jt + 1) * 128])
                c_cur.append(c_j)
            hT_cur = h0_s
            xt_cur = None

            for t in range(t_steps):
                p0 = (t % 3) * 32
                psz = psz_r[t % 2]

                # just-in-time phase-0a: block b_i = t//8 + 2, two matmuls
                # per step (jt chunk = (t%8)//2, kc pair = (t%8)%2)
                b_i = t // TB + 2
                sub = t % TB
                do_p0a = b_i < rb
                if do_p0a and sub == 0:
                    xt_cur = p0a_fetch_x(b_i)

                p_t = pin.tile([NL, J], bf16, tag="pt")
                bb, tt = divmod(t, TB)
                nc.scalar.dma_start(out=p_t[:, :],
                                    in_=p_d.ap()[bb, tt * NL:(tt + 1) * NL, :])

                # -- attention scores: s2 = A * hT (broadcast over m)
                s2 = [wk.tile([128, NL, M], bf16, tag=f"s2k{kc}",
                              name="s2_k") for kc in range(KC)]
                for kc in range(KC):
                    eng = nc.vector if kc == KC - 1 else nc.gpsimd
                    eng.tensor_tensor(
                        s2[kc][:, :, :], at_s[:, kc, :, :],
                        hT_cur[:, kc, :, None].broadcast_to([128, NL, M]),
                        mult)

                # -- Wh matmuls (kc 0-2) interleaved with the colsum
                for kc in range(KC):
                    if kc < KC - 1:
                        # Wh for the last chunk is held back to fill the
                        # PE idle window while the softmax chain runs
                        for jt in range(4):
                            nc.tensor.matmul(
                                psa_b[jt][p0:p0 + 16, :],
                                hT_cur[:, kc, :],
                                wh_s[:, kc, jt * 512:(jt + 1) * 512],
                                start=(kc == 0), stop=False)
                    if kc > 0:
                        # colsum for chunk kc-1 (s2 ready by now)
                        nc.tensor.matmul(psz[:, :, :], oc_s[:, :],
                                         s2[kc - 1][:, :, :],
                                         start=(kc == 1), stop=False)
                nc.tensor.matmul(psz[:, :, :], oc_s[:, :], s2[KC - 1][:, :, :],
                                 start=False, stop=True)

                # -- P inject + Wh kc3 (PE fillers while softmax runs)
                for jt in range(4):
                    nc.tensor.matmul(psa_b[jt][p0:p0 + 16, :],
                                     id_s[:, :],
                                     p_t[:, jt * 512:(jt + 1) * 512],
                                     start=False, stop=False)
                for jt in range(4):
                    nc.tensor.matmul(
                        psa_b[jt][p0:p0 + 16, :],
                        hT_cur[:, KC - 1, :],
                        wh_s[:, KC - 1, jt * 512:(jt + 1) * 512],
                        start=False, stop=False)
                # -- phase-0a fillers (2 matmuls of a future P block)
                if do_p0a:
                    jt0a = sub // 2
                    for kc in (0, 1) if sub % 2 == 0 else (2, 3):
                        p0a_matmul(xt_cur, jt0a, kc)
                    if sub % 2 == 1:
                        p0a_flush(b_i, jt0a)

                # -- softmax via tanh: e^s = (1+u)/(1-u), u = tanh(s/2)
                u_t = wk.tile([1, NL, M], f32, tag="u")
                nc.scalar.activation(u_t[:, :, :], psz[:, :, :], AF.Tanh,
                                     scale=half_inv_sqrt_h)
                den = wk.tile([1, NL, M], f32, tag="den")
                nc.vector.tensor_scalar(den[:, :, :], u_t[:, :, :],
                                        -1.0, 1.0, mult, add)
                rden = wk.tile([1, NL, M], f32, tag="rden")
                nc.vector.reciprocal_approx_fast(rden[:, :, :], den[:, :, :])
                r_t = wk.tile([1, NL, M], f32, tag="r")
                nc.vector.tensor_scalar(r_t[:, :, :], rden[:, :, :],
                                        2.0, -1.0, mult, add)
                ssum = wk.tile([1, NL, 1], f32, tag="ssum")
                nc.vector.tensor_reduce(ssum[:, :, :], r_t[:, :, :], AX, add)
                rsum = wk.tile([1, NL, 1], f32, tag="rsum")
                nc.vector.reciprocal_approx_fast(rsum[:, :, :], ssum[:, :, :])
                w_t = wk.tile([1, NL, M], bf16, tag="w")
                nc.vector.tensor_tensor(
                    w_t[:, :, :], r_t[:, :, :],
                    rsum[:, :, :].broadcast_to([1, NL, M]), mult)

                # -- w to partitions, block-diag stationary S_g
                s_g = wk.tile([128, NG, NL], bf16, tag="sg_w")
                for g in range(NG):
                    nc.tensor.transpose(wcol[g],
                                        w_t[0:1, g * 8:(g + 1) * 8, :],
                                        oc_s[0:1, 0:1])
                    nc.vector.tensor_tensor(
                        s_g[:, g, :], m0_s[:, g, :],
                        wcol[g].broadcast_to([128, NL]), mult)

                # -- attention contribution: psa += S_g^T . B_g, bank stops
                h_out = hop.tile([NL, H], bf16, tag="h")
                if t < t_steps - 1:
                    hT_next = stp.tile([128, KC, NL], bf16, tag="hT",
                                       name="hT_next")
                else:
                    hT_next = None
                c_nxt = [None] * 4
                sgs = []

                def h_finish(pj, psg):
                    pcs = slice(pj * 128, (pj + 1) * 128)
                    tc_t = wk.tile([NL, 128], bf16, tag=f"tc{pj}")
                    nc.scalar.activation(tc_t[:, :], c_nxt[pj][:, :], AF.Tanh)
                    nc.vector.tensor_tensor(h_out[:, pcs],
                                            psg[:, 256:384], tc_t[:, :],
                                            mult)
                    if hT_next is not None:
                        # eager per-chunk transpose: h cols [pj*128,..) are
                        # exactly hT chunk kc=pj
                        nc.tensor.transpose(tp[:, pj, 0:NL], h_out[:, pcs],
                                            id_s[:, :])
                        if pj in (0, 2):
                            nc.vector.tensor_copy(hT_next[:, pj, :],
                                                  tp[:, pj, 0:NL])
                        else:
                            nc.scalar.copy(hT_next[:, pj, :],
                                           tp[:, pj, 0:NL])

                for jt in range(4):
                    sl = slice(jt * 512, (jt + 1) * 512)
                    nc.tensor.matmul(psa_b[jt][p0:p0 + 16, :], s_g[:, 0, :],
                                     b_s[:, 0, sl], start=False, stop=False)
                    nc.tensor.matmul(psa_b[jt][p0:p0 + 16, :], s_g[:, 1, :],
                                     b_s[:, 1, sl], start=False, stop=True)
                    # bank jt complete: gates for H-slice jt
                    sg_t = wk.tile([NL, 384], bf16, tag=f"sg{jt}")
                    nc.scalar.activation(sg_t[:, :],
                                         psa_b[jt][p0:p0 + 16, 0:384],
                                         AF.Sigmoid)
                    tg_t = wk.tile([NL, 128], bf16, tag=f"tg{jt}")
                    nc.scalar.activation(tg_t[:, :],
                                         psa_b[jt][p0:p0 + 16, 384:512],
                                         AF.Tanh)
                    # c update for this slice
                    t1 = wk.tile([NL, 128], f32, tag=f"t1{jt}")
                    nc.gpsimd.tensor_tensor(t1[:, :], sg_t[:, 128:256],
                                            c_cur[jt][:, :], mult)
                    t2 = wk.tile([NL, 128], bf16, tag=f"t2{jt}")
                    nc.vector.tensor_tensor(t2[:, :], sg_t[:, 0:128],
                                            tg_t[:, :], mult)
                    c_nj = stp.tile([NL, 128], f32, tag=f"c{jt}", name="c_nj")
                    nc.vector.tensor_tensor(c_nj[:, :], t1[:, :],
                                            t2[:, :], add)
                    c_nxt[jt] = c_nj
                    sgs.append((jt, sg_t))
                    # tanh(c) lagged by one bank so the Act queue never
                    # stalls waiting for the c chain
                    if jt > 0:
                        h_finish(*sgs[jt - 1])
                h_finish(*sgs[3])

                nc.sync.dma_start(out=out_d.ap()[:, t, :], in_=h_out[:, :])

                hT_cur = hT_next
                c_cur = c_nxt

    nc.compile()
    return nc


def _perm_cols(w):
    """Permute gate columns: new bank jt = [i_jt | f_jt | o_jt | g_jt]."""
    # w: (..., 4H) with original layout [i(512) | f | o | g]
    w4 = w.reshape(w.shape[:-1] + (4, 4, 128))   # (..., gate, jt, col)
    return np.ascontiguousarray(
        np.moveaxis(w4, -3, -2).reshape(w.shape))  # (..., jt, gate, col)


def _stage_inputs(x, A, Wx, Wh, Wattn, b, t_steps=T):
    """Shard + lay out inputs per core (host-side numpy staging)."""
    rb = NL * t_steps // 128
    h0 = A.mean(axis=(2, 3)).astype(np.float32)          # (N, H)
    ident = np.eye(NL, dtype=BF16)
    ones_col = np.ones((128, 1), dtype=BF16)
    m0 = np.zeros((128, NG, NL), dtype=BF16)
    for g in range(NG):
        for i in range(8):
            m0[i * 16:(i + 1) * 16, g, g * 8 + i] = 1

    def wlay(w):
        return np.ascontiguousarray(
            _perm_cols(w).astype(BF16).reshape(KC, 128, J).transpose(1, 0, 2))

    wxs, whs, was = wlay(Wx), wlay(Wh), wlay(Wattn)
    bvec = np.ascontiguousarray(_perm_cols(b.astype(np.float32))
                                .reshape(1, J))

    maps = []
    for k in range(NCORES):
        ns = slice(k * NL, (k + 1) * NL)
        x_sh = x[ns, :t_steps].astype(BF16)              # (NL, t, D)
        xT = x_sh.transpose(2, 0, 1).reshape(KC, 128, NL, rb, TB)
        xs_st = np.ascontiguousarray(
            xT.transpose(3, 1, 0, 4, 2).reshape(rb, 128, KC, 128))
        A_sh = A[ns].reshape(NL, H, M).astype(BF16)
        at_st = np.ascontiguousarray(
            A_sh.transpose(1, 0, 2).reshape(KC, 128, NL, M)
            .transpose(1, 0, 2, 3))
        h0_sh = h0[ns]                                    # (NL, H)
        h0t = np.ascontiguousarray(
            h0_sh.T.astype(BF16).reshape(KC, 128, NL).transpose(1, 0, 2))
        m = {
            "xs": xs_st, "at": at_st, "wx": wxs, "wh": whs, "wa": was,
            "h0t": h0t, "c0": np.ascontiguousarray(h0_sh),
            "ident": ident, "ones_col": ones_col, "m0": m0,
        }
        if np.any(b != 0):
            m["bvec"] = bvec
        maps.append(m)
    return maps


def _get_nc(has_bias, t_steps=T):
    key = (has_bias, t_steps)
    if key not in _CACHE:
        _CACHE[key] = build(t_steps=t_steps, has_bias=has_bias)
    return _CACHE[key]


def run_cores(x, A, Wx, Wh, Wattn, b, t_steps=T, trace=False):
    from concourse.bass_utils import run_bass_kernel_spmd
    maps = _stage_inputs(x, A, Wx, Wh, Wattn, b, t_steps=t_steps)
    has_bias = "bvec" in maps[0]
    nc = _get_nc(has_bias, t_steps)
    res = run_bass_kernel_spmd(nc, maps, list(range(NCORES)), trace=trace)
    out = np.concatenate(
        [np.asarray(res.results[k]["out"], dtype=np.float32)
         for k in range(NCORES)], axis=0)
    return out, res


def kernel(x, A, Wx, Wh, Wattn, b):
    x = np.asarray(x, dtype=np.float32)
    A = np.asarray(A, dtype=np.float32)
    out, _ = run_cores(x, A,
                       np.asarray(Wx, dtype=np.float32),
                       np.asarray(Wh, dtype=np.float32),
                       np.asarray(Wattn, dtype=np.float32),
                       np.asarray(b, dtype=np.float32))
    return out
